# revision 25
# baseline (speedup 1.0000x reference)
"""Trainium2 Bass kernel for GPT2Attention with soft-threshold pruning.

Shapes: hidden_states [1, 2048, 1024], H=16 heads, head_dim=64.
Sharding: 2 heads per core across 8 cores (head parallel); c_attn columns and
c_proj rows split by head group; partial c_proj outputs summed on host.

Math per reference (no 1/sqrt(d) scaling):
    w   = q @ k^T                       (causal-masked to C=-1e4)
    w'  = C + (w - C) * sigmoid(10 w)
    a   = softmax(w', axis=-1)
    out = (a @ v) merged -> @ c_proj + b

Device-side formulation (all matmuls bf16, elementwise fp32):
  * shifted score p = w + 1e4 accumulated IN the QK^T matmul via two constant
    contraction rows (9984 + 16, both bf16-exact).
  * sigma(10w) = (1 + tanh(5w)) / 2 -- tanh and exp share one activation
    table ("exp_and_others"), so the scalar engine never reloads tables.
  * ws2 = (1 + tanh) * p = 2*(w - C)*sigma, fused on DVE via
    scalar_tensor_tensor; softmax numerator exp(0.5*ws2 - K) with a GLOBAL
    shift K = 10040 for all query blocks >= 1 (valid because every row
    q >= 128 has causal max w >= 5.1 >> 0.64, so the row max of w'' is
    within ~35 of K and nothing over/underflows).  Block 0 keeps a per-row
    max plus the masked-tail correction (exp(-m) * suffix sums of V).
  * scores are computed TRANSPOSED ([k, q] layout) so the exp output (bf16)
    feeds the A@V matmul directly as lhsT -- no transpose of the probability
    matrix is ever materialized.
  * denominator comes free as an extra ones-column in V; per-row normalization
    is applied to the [q, 64] head output before merging heads.
  * the whole kernel is software-pipelined: hs-transpose / V / QT,KT chunks
    interleave with the score units, and each unit's A@V is emitted one unit
    late so the PE never head-of-line blocks on the scalar engine's exp.
"""

import os
import sys

for _p in ("/opt/trn_rl_repo", "/root/.axon_site/_ro/trn_rl_repo"):
    if os.path.isdir(_p) and _p not in sys.path:
        sys.path.insert(0, _p)

import numpy as np

import concourse.bass as bass
import concourse.tile as tile
from concourse import bacc, mybir
from concourse.masks import make_identity

F32 = mybir.dt.float32
BF16 = mybir.dt.bfloat16
AF = mybir.ActivationFunctionType
ALU = mybir.AluOpType

S = 2048          # sequence length
D = 1024          # model dim
H = 16            # heads
HD = 64           # head dim
P = 128           # partitions
NB = S // P       # 16 seq blocks
NCORES = 8
HPC = H // NCORES  # 2 heads per core
KSH = 10040.0      # global softmax shift (covers |w| <= ~120)
BIAS_A = 9984.0    # bf16-exact pair summing to 1e4
BIAS_B = 16.0
VW = HD + 1        # 65: per-head V columns incl. ones (denominator) column
NG = 4             # q groups of 512

_CACHE = {}


def _build_nc():
    nc = bacc.Bacc(None, target_bir_lowering=False)

    hs_d = nc.dram_tensor("hs", [S, D], F32, kind="ExternalInput")
    wqkv_d = nc.dram_tensor("wqkv", [D, 3 * P], F32, kind="ExternalInput")
    bq_d = nc.dram_tensor("bq", [P, 1], F32, kind="ExternalInput")
    bk_d = nc.dram_tensor("bk", [P, 1], F32, kind="ExternalInput")
    bv_d = nc.dram_tensor("bv", [1, P], F32, kind="ExternalInput")
    wp_d = nc.dram_tensor("wp", [P, D], F32, kind="ExternalInput")
    out_d = nc.dram_tensor("out", [S, D], F32, kind="ExternalOutput")

    with tile.TileContext(nc) as tc:
        with (
            tc.tile_pool(name="const", bufs=1) as cpool,
            tc.tile_pool(name="pers", bufs=1) as qkpool,
            tc.tile_pool(name="hsload", bufs=6) as hlpool,
            tc.tile_pool(name="ws2", bufs=2) as wspool,
            tc.tile_pool(name="pexp", bufs=2) as ppool,
            tc.tile_pool(name="tch", bufs=3) as tpool,
            tc.tile_pool(name="stats", bufs=4) as stpool,
            tc.tile_pool(name="outsb", bufs=3) as opool,
            tc.tile_pool(name="psmm", bufs=2, space="PSUM") as ps_mm,
            tc.tile_pool(name="psacc", bufs=2, space="PSUM") as ps_acc,
            tc.tile_pool(name="psout", bufs=1, space="PSUM") as ps_out,
            tc.tile_pool(name="psbf", bufs=1, space="PSUM") as ps_bf,
        ):
            # -------- DMA issue order: first hs blocks 0..3, then weights,
            # then the rest of hs; wp (only needed for c_proj) last.
            hl = [hlpool.tile([P, D], F32, tag="hl", name=f"hl{_i}") for _i in range(NB)]
            for sb in range(4):
                nc.sync.dma_start(hl[sb], hs_d[P * sb : P * (sb + 1), :])
            w_sb = cpool.tile([P, D // P, 3 * P], F32)
            nc.sync.dma_start(w_sb, wqkv_d.rearrange("(o p) f -> p o f", p=P))
            bq_sb = cpool.tile([P, 1], F32)
            nc.sync.dma_start(bq_sb, bq_d[:])
            bk_sb = cpool.tile([P, 1], F32)
            nc.sync.dma_start(bk_sb, bk_d[:])
            bv_sb = cpool.tile([1, P], F32)
            nc.sync.dma_start(bv_sb, bv_d[:])
            for sb in range(4, NB):
                nc.sync.dma_start(hl[sb], hs_d[P * sb : P * (sb + 1), :])
            wp_sb = cpool.tile([P, D], F32)
            nc.sync.dma_start(wp_sb, wp_d[:])

            # -------- constants / conversions
            ident = cpool.tile([P, P], F32)
            make_identity(nc, ident)
            ident_bf = cpool.tile([P, P], BF16)
            make_identity(nc, ident_bf)
            ones1_bf = cpool.tile([1, P], BF16)
            nc.vector.memset(ones1_bf, 1.0)
            onesp_bf = cpool.tile([P, 1], BF16)
            nc.vector.memset(onesp_bf, 1.0)
            btanh = cpool.tile([P, 1], F32)   # tanh bias: -5*1e4
            nc.vector.memset(btanh, -5.0 * 1e4)
            bexp = cpool.tile([P, 1], F32)    # global exp shift: -K
            nc.vector.memset(bexp, -KSH)
            w_bf = cpool.tile([P, D // P, 3 * P], BF16)
            nc.vector.tensor_copy(w_bf, w_sb)
            bv_bf = cpool.tile([1, P], BF16)
            nc.vector.tensor_copy(bv_bf, bv_sb)
            wp_bf = cpool.tile([P, D], BF16)
            nc.vector.tensor_copy(wp_bf, wp_sb)

            # persistent per-core tensors (bf16); big memsets on gpsimd
            qt = [qkpool.tile([P, S], BF16, name=f"qt{h}") for h in range(HPC)]
            kt = [qkpool.tile([P, S], BF16, name=f"kt{h}") for h in range(HPC)]
            for n, t in enumerate(qt + kt):
                eng = nc.gpsimd if n % 2 == 0 else nc.vector
                eng.memset(t[HD:P, :], 0.0)
            for h in range(HPC):
                # +1e4 via two contraction rows at 32-aligned partitions 64/96
                nc.vector.memset(qt[h][HD : HD + 1, :], 1.0)
                nc.gpsimd.memset(qt[h][96:97, :], 1.0)
                nc.vector.memset(kt[h][HD : HD + 1, :], BIAS_A)
                nc.gpsimd.memset(kt[h][96:97, :], BIAS_B)
            hsT = qkpool.tile([P, D // P, S], BF16)  # [d%128, d//128, s]
            v_sb = qkpool.tile([P, NB, 2 * VW], BF16)
            for h in range(HPC):
                nc.gpsimd.memset(v_sb[:, :, VW * h + HD], 1.0)
            o_blk = [qkpool.tile([P, P], BF16, name=f"o{i}") for i in range(NB)]
            tail_bf = qkpool.tile([1, 2 * VW], BF16)  # V suffix sums (k>=128)

            # ---------------- emission helpers ----------------
            def emit_transp(sb):
                """hs[sb] -> hsT (PE fp32 transpose + bf16 copy)"""
                for dg in range(0, D // P, 4):
                    tp = ps_mm.tile([P, 512], F32, tag="mm")
                    for dc in range(dg, dg + 4):
                        nc.tensor.transpose(
                            tp[:, (dc - dg) * P : (dc - dg + 1) * P],
                            hl[sb][:, dc * P : (dc + 1) * P],
                            ident,
                        )
                    dst = hsT[:, dg : dg + 4, P * sb : P * (sb + 1)]
                    src = tp.rearrange("p (b f) -> p b f", b=4)
                    if dg == 0:
                        nc.scalar.copy(dst, src)
                    else:
                        nc.vector.tensor_copy(dst, src)

            def emit_v(sb):
                vp = ps_acc.tile([P, P], F32, tag="acc")
                for dc in range(D // P):
                    nc.tensor.matmul(
                        vp,
                        lhsT=hsT[:, dc, P * sb : P * (sb + 1)],
                        rhs=w_bf[:, dc, 2 * P : 3 * P],
                        start=(dc == 0),
                        stop=False,
                    )
                nc.tensor.matmul(
                    vp, lhsT=ones1_bf, rhs=bv_bf, start=False, stop=True
                )
                for h in range(HPC):
                    nc.vector.tensor_copy(
                        v_sb[:, sb, VW * h : VW * h + HD],
                        vp[:, HD * h : HD * (h + 1)],
                    )

            def emit_qtkt(sc):
                """one 512-col s-chunk of QT and KT (both heads + biases)"""
                for off, dst, b_ap in ((0, qt, bq_sb), (P, kt, bk_sb)):
                    qp = ps_mm.tile([P, 512], F32, tag="mm")
                    for dc in range(D // P):
                        nc.tensor.matmul(
                            qp,
                            lhsT=w_bf[:, dc, off : off + P],
                            rhs=hsT[:, dc, 512 * sc : 512 * (sc + 1)],
                            start=(dc == 0),
                            stop=(dc == D // P - 1),
                        )
                    for h in range(HPC):
                        nc.vector.tensor_scalar(
                            dst[h][0:HD, 512 * sc : 512 * (sc + 1)],
                            qp[HD * h : HD * (h + 1)],
                            b_ap[HD * h : HD * (h + 1)],
                            None,
                            ALU.add,
                        )

            def emit_ssuf():
                ssuf_ps = ps_out.tile([1, 2 * VW], F32, tag="po")
                for sb in range(1, NB):
                    nc.tensor.matmul(
                        ssuf_ps,
                        lhsT=onesp_bf,
                        rhs=v_sb[:, sb, :],
                        start=(sb == 1),
                        stop=(sb == NB - 1),
                    )
                nc.vector.tensor_copy(tail_bf, ssuf_ps)

            def emit_scores(g, h):
                """transposed scores for q-group g, head h -> pexpT (bf16)"""
                q0 = max(P, 512 * g)
                qw = 512 * (g + 1) - q0
                njs = 4 * (g + 1)
                pexpT = ppool.tile([P, 16, 512], BF16, tag="pexpT")
                for jg in range(0, njs, 4):
                    ws2 = wspool.tile([P, 4, 512], F32, tag="ws2")
                    for jp in (jg, jg + 2):
                        offs = [max(0, P * (jp + u) - q0) for u in (0, 1)]
                        pj2 = ps_mm.tile([P, 1024], F32, tag="mm")
                        tj2 = tpool.tile([P, 1024], F32, tag="tch")
                        for u in (0, 1):
                            oj = offs[u]
                            nc.tensor.matmul(
                                pj2[:, 512 * u : 512 * u + qw - oj],
                                lhsT=kt[h][:, P * (jp + u) : P * (jp + u + 1)],
                                rhs=qt[h][:, q0 + oj : q0 + qw],
                                start=True,
                                stop=True,
                            )
                        same = offs[0] == offs[1]
                        if same:  # one tanh instruction covers both chunks
                            w0 = qw - offs[0]
                            nc.scalar.activation(
                                tj2.rearrange("p (b f) -> p b f", b=2)[:, :, :w0],
                                pj2.rearrange("p (b f) -> p b f", b=2)[:, :, :w0],
                                AF.Tanh, scale=5.0, bias=btanh,
                            )
                        else:
                            for u in (0, 1):
                                oj = offs[u]
                                nc.scalar.activation(
                                    tj2[:, 512 * u : 512 * u + qw - oj],
                                    pj2[:, 512 * u : 512 * u + qw - oj],
                                    AF.Tanh, scale=5.0, bias=btanh,
                                )
                        for u in (0, 1):
                            j = jp + u
                            oj = offs[u]
                            wj = qw - oj
                            if P * (j + 1) > q0 + oj:  # diagonal: mask k > q
                                nc.gpsimd.affine_select(
                                    out=tj2[:, 512 * u : 512 * u + wj],
                                    in_=tj2[:, 512 * u : 512 * u + wj],
                                    pattern=[[1, wj]],
                                    channel_multiplier=-1,
                                    base=q0 + oj - P * j,
                                    compare_op=ALU.is_ge,
                                    fill=-1.0,
                                )
                        if same:
                            w0 = qw - offs[0]
                            nc.vector.scalar_tensor_tensor(
                                ws2[:, jp - jg : jp - jg + 2, offs[0] : qw],
                                in0=tj2.rearrange("p (b f) -> p b f", b=2)[:, :, :w0],
                                scalar=1.0,
                                in1=pj2.rearrange("p (b f) -> p b f", b=2)[:, :, :w0],
                                op0=ALU.add, op1=ALU.mult,
                            )
                        else:
                            for u in (0, 1):
                                oj = offs[u]
                                nc.vector.scalar_tensor_tensor(
                                    ws2[:, jp - jg + u, oj:qw],
                                    in0=tj2[:, 512 * u : 512 * u + qw - oj],
                                    scalar=1.0,
                                    in1=pj2[:, 512 * u : 512 * u + qw - oj],
                                    op0=ALU.add, op1=ALU.mult,
                                )
                    # exp over the full rectangle; entries left of each oj are
                    # stale buffer contents, finite, and never read by A@V.
                    nc.scalar.activation(
                        pexpT[:, jg : jg + 4, :qw], ws2[:, :, :qw], AF.Exp,
                        scale=0.5, bias=bexp,
                    )
                return pexpT

            def emit_av(g, h, pexpT):
                q0 = max(P, 512 * g)
                qw = 512 * (g + 1) - q0
                for qb in range(qw // P):
                    i = (q0 + P * qb) // P
                    o_ps = ps_acc.tile([P, VW], F32, tag="acc")
                    for j in range(i + 1):
                        nc.tensor.matmul(
                            o_ps,
                            lhsT=pexpT[:, j, P * qb : P * (qb + 1)],
                            rhs=v_sb[:, j, VW * h : VW * (h + 1)],
                            start=(j == 0),
                            stop=(j == i),
                        )
                    recip = stpool.tile([P, 1], F32, tag="recip")
                    nc.vector.reciprocal(recip, o_ps[:, HD : HD + 1])
                    nc.vector.tensor_scalar_mul(
                        o_blk[i][:, HD * h : HD * (h + 1)], o_ps[:, :HD], recip
                    )
                    if h == 1:
                        cproj(i)

            def emit_b0_scores(h):
                """block 0, untransposed, per-row max; returns tiles for AV"""
                p0 = ps_mm.tile([P, 512], F32, tag="mm")
                nc.tensor.matmul(
                    p0[:, :P], lhsT=qt[h][:, :P], rhs=kt[h][:, :P],
                    start=True, stop=True,
                )
                t0 = tpool.tile([P, 512], F32, tag="tch")
                nc.scalar.activation(
                    t0[:, :P], p0[:, :P], AF.Tanh, scale=5.0, bias=btanh
                )
                nc.gpsimd.affine_select(
                    out=t0[:, :P], in_=t0[:, :P], pattern=[[-1, P]],
                    channel_multiplier=1, base=0,
                    compare_op=ALU.is_ge, fill=-1.0,
                )
                ws2_0 = stpool.tile([P, P], F32, tag="ws20")
                nc.vector.scalar_tensor_tensor(
                    ws2_0, in0=t0[:, :P], scalar=1.0, in1=p0[:, :P],
                    op0=ALU.add, op1=ALU.mult,
                )
                m2 = stpool.tile([P, 1], F32, tag="m2")
                nc.vector.tensor_reduce(m2, ws2_0, mybir.AxisListType.X, ALU.max)
                negm = stpool.tile([P, 1], F32, tag="negm")
                nc.vector.tensor_scalar_mul(negm, m2, -0.5)
                pexp0 = stpool.tile([P, P], BF16, tag="pexp0")
                nc.scalar.activation(pexp0, ws2_0, AF.Exp, scale=0.5, bias=negm)
                e_sb = stpool.tile([P, 1], F32, tag="e_sb")
                nc.scalar.activation(e_sb, m2, AF.Exp, scale=-0.5)
                epad = stpool.tile([P, P], F32, tag="epad")
                nc.vector.memset(epad, 0.0)
                nc.vector.tensor_copy(epad[:, 0:1], e_sb)
                eT_ps = ps_out.tile([P, P], F32, tag="po")
                nc.tensor.transpose(eT_ps, epad, ident)
                eT_bf = stpool.tile([1, P], BF16, tag=f"eT_bf{h}")
                nc.vector.tensor_copy(eT_bf, eT_ps[0:1, :])
                pT_ps = ps_bf.tile([P, P], BF16, tag="pobf")
                nc.tensor.transpose(pT_ps, pexp0, ident_bf)
                pT0 = stpool.tile([P, P], BF16, tag=f"pT0{h}")
                nc.vector.tensor_copy(pT0, pT_ps)
                return pT0, eT_bf

            def emit_b0_av(h, pT0, eT_bf):
                o_ps = ps_acc.tile([P, VW], F32, tag="acc")
                nc.tensor.matmul(
                    o_ps, lhsT=pT0, rhs=v_sb[:, 0, VW * h : VW * (h + 1)],
                    start=True, stop=False,
                )
                nc.tensor.matmul(
                    o_ps, lhsT=eT_bf, rhs=tail_bf[:, VW * h : VW * (h + 1)],
                    start=False, stop=True,
                )
                recip = stpool.tile([P, 1], F32, tag="recip")
                nc.vector.reciprocal(recip, o_ps[:, HD : HD + 1])
                nc.vector.tensor_scalar_mul(
                    o_blk[0][:, HD * h : HD * (h + 1)], o_ps[:, :HD], recip
                )

            def cproj(i):
                otp = ps_bf.tile([P, P], BF16, tag="pobf")
                nc.tensor.transpose(otp, o_blk[i], ident_bf)
                ot_bf = opool.tile([P, P], BF16, tag="ot_bf")
                nc.vector.tensor_copy(ot_bf, otp)
                y_sb = opool.tile([P, D], F32, tag="y_sb")
                for nch in range(D // 512):
                    yp = ps_out.tile([P, 512], F32, tag="po")
                    nc.tensor.matmul(
                        yp,
                        lhsT=ot_bf,
                        rhs=wp_bf[:, 512 * nch : 512 * (nch + 1)],
                        start=True,
                        stop=True,
                    )
                    if nch == 0:
                        nc.scalar.copy(y_sb[:, 512 * nch : 512 * (nch + 1)], yp)
                    else:
                        nc.vector.tensor_copy(
                            y_sb[:, 512 * nch : 512 * (nch + 1)], yp
                        )
                nc.sync.dma_start(out_d[P * i : P * (i + 1), :], y_sb)

            # ---------------- pipelined emission ----------------
            for sb in range(4):
                emit_transp(sb)
                emit_v(sb)
            emit_qtkt(0)

            for sb in range(4, 8):
                emit_transp(sb)
                emit_v(sb)
            px_a = emit_scores(0, 0)          # U1
            emit_qtkt(1)

            for sb in range(8, 12):
                emit_transp(sb)
                emit_v(sb)
            px_b = emit_scores(0, 1)          # U2
            emit_av(0, 0, px_a)               # AV(U1)
            emit_qtkt(2)

            for sb in range(12, 16):
                emit_transp(sb)
                emit_v(sb)
            px_a = emit_scores(1, 0)          # U3
            emit_av(0, 1, px_b)               # AV(U2)
            emit_ssuf()
            b0s = [emit_b0_scores(h) for h in range(HPC)]
            emit_qtkt(3)

            px_b = emit_scores(1, 1)          # U4
            emit_av(1, 0, px_a)               # AV(U3)
            for h in range(HPC):
                emit_b0_av(h, *b0s[h])

            px_a = emit_scores(2, 0)          # U5
            emit_av(1, 1, px_b)               # AV(U4)
            cproj(0)

            px_b = emit_scores(2, 1)          # U6
            emit_av(2, 0, px_a)               # AV(U5)

            px_a = emit_scores(3, 0)          # U7
            emit_av(2, 1, px_b)               # AV(U6)

            px_b = emit_scores(3, 1)          # U8
            emit_av(3, 0, px_a)               # AV(U7)

            emit_av(3, 1, px_b)               # AV(U8)

    nc.compile()
    return nc


def _get_nc():
    if "nc" not in _CACHE:
        _CACHE["nc"] = _build_nc()
    return _CACHE["nc"]


def kernel(hidden_states, c_attn_w, c_attn_b, c_proj_w, c_proj_b):
    from concourse.bass_utils import run_bass_kernel_spmd

    hs = np.ascontiguousarray(np.asarray(hidden_states, np.float32).reshape(S, D))
    caw = np.asarray(c_attn_w, np.float32)
    cab = np.asarray(c_attn_b, np.float32)
    cpw = np.asarray(c_proj_w, np.float32)
    cpb = np.asarray(c_proj_b, np.float32)

    in_maps = []
    for c in range(NCORES):
        heads = [HPC * c + h for h in range(HPC)]
        qcols = [caw[:, HD * h : HD * (h + 1)] for h in heads]
        kcols = [caw[:, D + HD * h : D + HD * (h + 1)] for h in heads]
        vcols = [caw[:, 2 * D + HD * h : 2 * D + HD * (h + 1)] for h in heads]
        wqkv = np.ascontiguousarray(np.concatenate(qcols + kcols + vcols, axis=1))
        bq = np.concatenate([cab[HD * h : HD * (h + 1)] for h in heads])
        bk = np.concatenate([cab[D + HD * h : D + HD * (h + 1)] for h in heads])
        bv = np.concatenate([cab[2 * D + HD * h : 2 * D + HD * (h + 1)] for h in heads])
        wp = np.ascontiguousarray(cpw[P * c : P * (c + 1), :])
        in_maps.append(
            {
                "hs": hs,
                "wqkv": wqkv,
                "bq": np.ascontiguousarray(bq.reshape(P, 1)),
                "bk": np.ascontiguousarray(bk.reshape(P, 1)),
                "bv": np.ascontiguousarray(bv.reshape(1, P)),
                "wp": wp,
            }
        )

    nc = _get_nc()
    res = run_bass_kernel_spmd(nc, in_maps, core_ids=list(range(NCORES)))
    out = np.zeros((S, D), np.float64)
    for c in range(NCORES):
        out += res.results[c]["out"].astype(np.float64)
    out = out.astype(np.float32) + cpb[None, :].astype(np.float32)
    return out.reshape(1, S, D)
